# revision 32
# baseline (speedup 1.0000x reference)
"""Trainium2 Bass kernel for a dense-transformer attention block.

Reference semantics (T=2048, D=2048, 16 heads, d_h=128):
    h = RMSNorm(x) * ln_w
    q,k,v = h @ W{q,k,v}.T  -> (n_h, T, d_h);  RoPE(q, k)
    att = softmax(causal(q k^T / sqrt(d_h))) @ v
    out = x + att @ Wo.T          (attention_mask is all-ones per spec)

Distribution: head-parallel over 8 cores (2 heads/core) for QKV+attention;
the output is SEQUENCE-sharded: one AllToAll redistributes att^T so each
core holds all 2048 att rows for its own 256 output timesteps, then each
core runs the FULL Wo against its timestep slice (~8x fewer collective
bytes than all-gathering att).

Engine plan per core:
  r-chain  RMSNorm row scales r[t] from a row-major x copy; squares split
           between ScalarE (Act Square + accum_out) and DVE
           (tensor_tensor_reduce), x_rows loads paced on the Act queue so
           they never block the xT stream; bit-trick rsqrt Newton on DVE in
           [128,8] column layout (two halves); r broadcast across
           partitions via a ones-column PE matmul, folded into bf16-loaded
           RoPE tables (f32 result).
  QKV      per 512-block B: bf16 matmuls (ln_w folded into weights on
           host); Q/K PSUM drained raw to SBUF so the PE never waits on r;
           RoPE applied in-place on DVE with a sign-folded sin table and a
           partition-half-swap SBUF DMA.
  att      causal attention, scores transposed (S^T[j,i]); i-range sliced
           to skip fully-masked diagonal work; softmax row-sums on the PE
           via a ones-vector matmul; exp on ScalarE; 1/sum broadcast via
           ones-column PE matmul.  gpsimd carries ONLY mask setup and
           collective triggers (a trigger blocks the gpsimd queue).
  Wo       out[i,:] = att_all^T @ Wo, weights streamed during attention on
           spread DMA queues; residual added from a bf16 x slice preloaded
           into dead K_sb space; per-tile drains inline with the last
           accumulation pass.
Host assembles out = concat(out_rows, axis=0).
"""

import contextlib
import math

import numpy as np

EPS = 1e-5
NEG = -1.0e30

CFG_FULL = dict(T=2048, D=2048, n_cores=8, heads_per_core=2)


# --------------------------------------------------------------------------
# device program
# --------------------------------------------------------------------------
def build_nc(T, D, n_cores, heads_per_core):
    import concourse.mybir as mybir
    import concourse.tile as tile
    from concourse import bacc

    DH = 128                      # head dim (hard-wired into layout)
    P = 128                       # partitions
    NH = heads_per_core
    DL = NH * DH                  # local width (q/k/v columns per core)
    KC = D // P                   # 128-contraction chunks over d_model
    NTS = T // P                  # 128-wide t subtiles
    TS_C = T // n_cores           # output timesteps per core (256)
    RG = 4                        # Wo column groups of 512
    f32 = mybir.dt.float32
    bf16 = mybir.dt.bfloat16
    i32 = mybir.dt.int32

    nc = bacc.Bacc("TRN2", target_bir_lowering=False, debug=False,
                   num_devices=n_cores)

    # ---- I/O ----
    xT = nc.dram_tensor("xT", [D, T], bf16, kind="ExternalInput").ap()
    x_rows = nc.dram_tensor("x_rows", [T, D], bf16, kind="ExternalInput").ap()
    x_seq = nc.dram_tensor("x_seq", [TS_C, D], bf16, kind="ExternalInput").ap()
    wq_t = nc.dram_tensor("wq_t", [P, KC * DL], bf16, kind="ExternalInput").ap()
    wk_t = nc.dram_tensor("wk_t", [P, KC * DL], bf16, kind="ExternalInput").ap()
    wv_t = nc.dram_tensor("wv_t", [P, KC * DL], bf16, kind="ExternalInput").ap()
    wo_t = nc.dram_tensor("wo_t", [P, KC * RG * 512], bf16,
                          kind="ExternalInput").ap()
    cosT = nc.dram_tensor("cosT", [DH, T], bf16, kind="ExternalInput").ap()
    sinmT = nc.dram_tensor("sinmT", [DH, T], bf16, kind="ExternalInput").ap()
    out_rows = nc.dram_tensor("out_rows", [TS_C, D], f32,
                              kind="ExternalOutput").ap()

    Act = mybir.ActivationFunctionType
    Alu = mybir.AluOpType
    inv_sqrt_dh = 1.0 / math.sqrt(DH)
    MAGIC = 0x5F3759DF
    groups = [list(range(n_cores))]

    with tile.TileContext(nc) as tc, \
            tc.tile_pool(name="persist", bufs=1) as persist:
        # ---------------- long-lived tensors ----------------
        Q_sb = persist.tile([P, NH, T], bf16, tag="Q_sb")
        K_sb = persist.tile([P, NH, T], bf16, tag="K_sb")
        V_sb = persist.tile([P, NTS, DL], bf16, tag="V_sb")
        rcol_sb = persist.tile([P, NTS], f32, tag="rcol_sb")
        rrow_sb = persist.tile([1, T], bf16, tag="rrow_sb")
        ones_bf = persist.tile([P, 1], bf16, tag="ones_bf")
        ones_row = persist.tile([1, P], bf16, tag="ones_row")
        mask_sb = persist.tile([P, P], f32, tag="mask_sb")
        warm_sb = persist.tile([P, 128], bf16, tag="warm_sb")

        nc.vector.memset(ones_bf[:], 1.0)
        nc.vector.memset(ones_row[:], 1.0)
        nc.vector.memset(warm_sb[:], 0.0)

        with tc.tile_pool(name="dram", bufs=1, space="DRAM") as dram_pool:
            a2a_in = dram_pool.tile([n_cores, DL, TS_C], bf16, tag="a2ai",
                                    name="a2a_in")
            a2a_out = dram_pool.tile([n_cores, DL, TS_C], bf16, tag="a2ao",
                                     name="a2a_out")
            dummy_i = dram_pool.tile([n_cores, 128], bf16, tag="dmi",
                                     name="dummy_i")
            dummy_o = dram_pool.tile([n_cores, 128], bf16, tag="dmo",
                                     name="dummy_o")

            # gpsimd queue carries ONLY the mask setup + collective
            # triggers: a collective trigger blocks the gpsimd queue until
            # the collective completes.
            nc.gpsimd.memset(mask_sb[:], 0.0)
            # keep (0) where j <= i within the diagonal 128x128 block:
            # j = base + p, i = base + f  ->  keep f - p >= 0
            nc.gpsimd.affine_select(
                out=mask_sb[:], in_=mask_sb[:],
                pattern=[[1, P]], channel_multiplier=-1, base=0,
                compare_op=Alu.is_ge, fill=NEG)
            # warm the collective path (absorbs the cross-core entry
            # barrier + first-op setup while the DMA-bound prologue runs)
            nc.sync.dma_start(dummy_i[:], warm_sb[:n_cores, :])
            nc.gpsimd.collective_compute(
                "AllToAll", Alu.bypass, replica_groups=groups,
                ins=[dummy_i[:].opt()], outs=[dummy_o[:].opt()])

            # PE warmup: back-to-back dummy matmuls so the HAM clock gate
            # opens before the real work arrives
            with tc.tile_pool(name="warm_ps", bufs=1, space="PSUM") as wmps:
                wps = wmps.tile([P, 128], f32, tag="wm")
                for _ in range(24):
                    nc.tensor.matmul(wps[:], warm_sb[:], warm_sb[:],
                                     start=True, stop=True)

            with contextlib.ExitStack() as stk_wo:
                wopool = stk_wo.enter_context(tc.tile_pool(name="wo", bufs=6))
                stk0 = stk_wo.enter_context(contextlib.ExitStack())
                _p = lambda *a, **kw: stk0.enter_context(tc.tile_pool(*a, **kw))
                wpool = _p(name="wqkv", bufs=1)
                cspool = _p(name="cs_raw", bufs=1)
                xpool = _p(name="xk", bufs=1)
                xrpool = _p(name="xrow", bufs=3)
                sqpool = _p(name="sqs", bufs=1)
                rpool = _p(name="rtmp", bufs=1)
                ropool = _p(name="rope", bufs=2)
                ptpool = _p(name="ptp", bufs=3)
                finpool = _p(name="fin", bufs=2)
                pps = _p(name="proj_ps", bufs=3, space="PSUM")
                stps = _p(name="st_ps", bufs=2, space="PSUM")
                avps = _p(name="av_ps", bufs=1, space="PSUM")
                rowps = _p(name="row_ps", bufs=1, space="PSUM")
                bcps = _p(name="bc_ps", bufs=1, space="PSUM")

                # ---- initial loads: xT stream on sync; x_rows paced on the
                # Act queue (so squares never block xT); tables on Act queue
                wq_sb = wpool.tile([P, KC, DL], bf16, tag="wq")
                wk_sb = wpool.tile([P, KC, DL], bf16, tag="wk")
                wv_sb = wpool.tile([P, KC, DL], bf16, tag="wv")
                xk = [xpool.tile([P, T], bf16, tag=f"xk{kc}", name=f"xk{kc}")
                      for kc in range(KC)]
                cos_bf = cspool.tile([P, T], bf16, tag="cosb")
                sinm_bf = cspool.tile([P, T], bf16, tag="sinb")
                nc.scalar.dma_start(cos_bf[:], cosT)
                nc.scalar.dma_start(sinm_bf[:], sinmT)
                nc.sync.dma_start(wq_sb[:], wq_t.rearrange("p (kc j) -> p kc j", j=DL))
                # B-major x loading: block 0's slices land first so the
                # first QKV block is fed after ~2MB instead of ~8MB
                for B in range(4):
                    tb = slice(512 * B, 512 * B + 512)
                    for kc in range(KC):
                        nc.sync.dma_start(xk[kc][:, tb],
                                          xT[P * kc:P * (kc + 1), tb])
                nc.sync.dma_start(wk_sb[:], wk_t.rearrange("p (kc j) -> p kc j", j=DL))
                nc.sync.dma_start(wv_sb[:], wv_t.rearrange("p (kc j) -> p kc j", j=DL))
                cos_r = cspool.tile([P, T], f32, tag="cos")
                sinm_r = cspool.tile([P, T], f32, tag="sin")

                # ---- RMSNorm row scales r = rsqrt(mean(x^2)+eps) ----
                rs_raw = rpool.tile([P, NTS], f32, tag="rs_raw")
                rcol_bf = rpool.tile([P, NTS], bf16, tag="rcol_bf")
                mcol = rpool.tile([P, NTS], f32, tag="mcol")
                ri = rpool.tile([P, NTS], i32, tag="ri")
                tn = rpool.tile([P, NTS], f32, tag="tn")
                HC = NTS // 2  # chunks per half

                def r_sums(half):
                    hsl = slice(HC * half, HC * (half + 1))
                    for ch in range(HC * half, HC * (half + 1)):
                        xr = xrpool.tile([P, D], bf16, tag="xr",
                                         name=f"xr{ch}")
                        nc.scalar.dma_start(xr[:],
                                            x_rows[P * ch:P * (ch + 1), :])
                        acc = rs_raw[:, ch:ch + 1]
                        scr = sqpool.tile([P, D], bf16, tag="scrA")
                        nc.scalar.activation(scr[:], xr[:], Act.Square,
                                             accum_out=acc)
                    # r = rsqrt(sum/D + eps): bit-trick seed + 2 Newton
                    nc.vector.tensor_scalar(mcol[:, hsl], rs_raw[:, hsl],
                                            1.0 / D, EPS, Alu.mult, Alu.add)
                    nc.vector.tensor_scalar(ri[:, hsl],
                                            mcol[:, hsl].bitcast(i32), 1,
                                            None, Alu.arith_shift_right)
                    nc.vector.tensor_scalar(ri[:, hsl], ri[:, hsl], -1, MAGIC,
                                            Alu.mult, Alu.add)
                    rrv = ri[:, hsl].bitcast(f32)
                    for _ in range(2):
                        nc.vector.tensor_tensor(tn[:, hsl], rrv, rrv, Alu.mult)
                        nc.vector.tensor_tensor(tn[:, hsl], tn[:, hsl],
                                                mcol[:, hsl], Alu.mult)
                        nc.vector.tensor_scalar(tn[:, hsl], tn[:, hsl], -0.5,
                                                1.5, Alu.mult, Alu.add)
                        nc.vector.tensor_tensor(rrv, rrv, tn[:, hsl], Alu.mult)
                    nc.vector.tensor_copy(rcol_sb[:, hsl], rrv)
                    nc.vector.tensor_copy(rcol_bf[:, hsl], rrv)
                    for ch in range(HC * half, HC * (half + 1)):
                        nc.sync.dma_start(rrow_sb[0:1, P * ch:P * (ch + 1)],
                                          rcol_bf[:, ch:ch + 1])

                def r_tables(half):
                    # broadcast r across partitions on the PE (ones column x
                    # r row) and fold into the RoPE tables
                    for s in range(2 * half, 2 * half + 2):
                        tsl = slice(512 * s, 512 * (s + 1))
                        rps = bcps.tile([P, 512], f32, tag="bc",
                                        name=f"rbc{s}")
                        nc.tensor.matmul(rps[:], ones_row[:],
                                         rrow_sb[0:1, tsl],
                                         start=True, stop=True)
                        nc.vector.tensor_tensor(cos_r[:, tsl], cos_bf[:, tsl],
                                                rps[:], Alu.mult)
                        nc.vector.tensor_tensor(sinm_r[:, tsl],
                                                sinm_bf[:, tsl],
                                                rps[:], Alu.mult)

                # Wo weight chunks: stream during attention, spread queues
                wo_view = wo_t.rearrange("p (kc rg j) -> p kc rg j",
                                         kc=KC, rg=RG)
                wo_chunks = [None] * KC
                _wo_engs = [nc.sync, nc.scalar]

                def load_woc(kc):
                    woc = wopool.tile([P, RG, 512], bf16, tag="woc",
                                      name=f"woc{kc}")
                    _wo_engs[kc % 2].dma_start(woc[:], wo_view[:, kc, :, :])
                    wo_chunks[kc] = woc

                # ---- per-block QKV matmuls (PE decoupled from r) ----
                def qkv_mms(B):
                    tb = slice(512 * B, 512 * B + 512)
                    for qk, (w_sb, dst) in enumerate(((wq_sb, Q_sb),
                                                      (wk_sb, K_sb))):
                        for h in range(NH):
                            hs = slice(DH * h, DH * (h + 1))
                            qp = pps.tile([P, 512], f32, tag="proj",
                                          name=f"p{B}_{qk}_{h}")
                            for kc in range(KC):
                                nc.tensor.matmul(qp[:], w_sb[:, kc, hs],
                                                 xk[kc][:, tb],
                                                 start=(kc == 0),
                                                 stop=(kc == KC - 1))
                            # raw drain; RoPE comes later, in place
                            nc.vector.tensor_copy(dst[:, h, tb], qp[:])
                    for ts in range(4):
                        i = 4 * B + ts
                        tsl = slice(512 * B + P * ts, 512 * B + P * (ts + 1))
                        vp = pps.tile([P, 512], f32, tag="proj",
                                      name=f"pv{B}_{ts}")
                        for kc in range(KC):
                            nc.tensor.matmul(vp[:, :DL], xk[kc][:, tsl],
                                             wv_sb[:, kc, :], start=(kc == 0),
                                             stop=(kc == KC - 1))
                        nc.vector.tensor_scalar_mul(V_sb[:, i, :], vp[:, :DL],
                                                    rcol_sb[:, i:i + 1])

                def rope(B):
                    tb = slice(512 * B, 512 * B + 512)
                    for dst in (Q_sb, K_sb):
                        for h in range(NH):
                            # dst = dst*cos_r + swap64(dst*sinm_r), in place
                            tmp = ropool.tile([P, 512], bf16, tag="tmp")
                            tmp2 = ropool.tile([P, 512], bf16, tag="tmp2")
                            nc.vector.tensor_tensor(tmp[:], dst[:, h, tb],
                                                    sinm_r[:, tb], Alu.mult)
                            nc.sync.dma_start(tmp2[0:64, :], tmp[64:128, :])
                            nc.sync.dma_start(tmp2[64:128, :], tmp[0:64, :])
                            nc.vector.tensor_tensor(dst[:, h, tb],
                                                    dst[:, h, tb],
                                                    cos_r[:, tb], Alu.mult)
                            nc.vector.tensor_tensor(dst[:, h, tb],
                                                    dst[:, h, tb], tmp2[:],
                                                    Alu.add)

                def att_block(B):
                    for h in range(NH):
                        hs = slice(DH * h, DH * (h + 1))
                        av = avps.tile([P, 512], f32, tag="av",
                                       name=f"av{B}_{h}")
                        ssum = rowps.tile([1, 512], f32, tag="row",
                                          name=f"ss{B}_{h}")
                        Jmax = 4 * B + 3
                        for J in range(Jmax + 1):
                            r = J - 4 * B  # >=0 on the diagonal 512-block
                            lo = max(0, 128 * r)  # live i-range start
                            isl = slice(512 * B + lo, 512 * B + 512)
                            st = stps.tile([P, 512], f32, tag="st",
                                           name=f"st{B}_{h}_{J}")
                            nc.tensor.matmul(st[:, lo:],
                                             K_sb[:, h, P * J:P * (J + 1)],
                                             Q_sb[:, h, isl],
                                             start=True, stop=True)
                            if r >= 0:
                                nc.vector.tensor_tensor(
                                    st[:, lo:lo + P], st[:, lo:lo + P],
                                    mask_sb[:], Alu.add)
                            pt = ptpool.tile([P, 512], bf16, tag="pt")
                            nc.scalar.activation(pt[:, lo:], st[:, lo:],
                                                 Act.Exp, scale=inv_sqrt_dh)
                            nc.tensor.matmul(av[:, lo:], V_sb[:, J, hs],
                                             pt[:, lo:], start=(J == 0),
                                             stop=(J == Jmax),
                                             skip_group_check=True)
                            nc.tensor.matmul(ssum[:, lo:], ones_bf[:],
                                             pt[:, lo:], start=(J == 0),
                                             stop=(J == Jmax),
                                             skip_group_check=True)
                        rinv = finpool.tile([1, 512], f32, tag="rinv")
                        nc.vector.reciprocal_approx_fast(rinv[:], ssum[:])
                        rinv_bf = finpool.tile([1, 512], bf16, tag="rinvb")
                        nc.vector.tensor_copy(rinv_bf[:], rinv[:])
                        rb = bcps.tile([P, 512], f32, tag="bc",
                                       name=f"rb{B}_{h}")
                        nc.tensor.matmul(rb[:], ones_row[:], rinv_bf[:],
                                         start=True, stop=True)
                        rb_sb = finpool.tile([P, 512], f32, tag="rbsb")
                        nc.scalar.activation(rb_sb[:], rb[:], Act.Copy)
                        att = finpool.tile([P, 512], bf16, tag="att")
                        nc.vector.tensor_tensor(att[:], av[:], rb_sb[:],
                                                Alu.mult)
                        # store to the AllToAll source: dest core 2B+c' gets
                        # i-window att[:, 256c' : 256c'+256]
                        nc.sync.dma_start(
                            a2a_in[2 * B:2 * B + 2, DH * h:DH * (h + 1), :]
                            .rearrange("c p i -> p c i"),
                            att[:])

                r_sums(0)
                r_sums(1)
                qkv_mms(0)
                r_tables(0)
                r_tables(1)
                rope(0)
                qkv_mms(1)
                rope(1)
                load_woc(0)
                load_woc(1)
                att_block(0)
                qkv_mms(2)
                rope(2)
                for kc in range(2, 6):
                    load_woc(kc)
                att_block(1)
                qkv_mms(3)
                rope(3)
                for kc in range(6, 11):
                    load_woc(kc)
                att_block(2)
                for kc in range(11, KC):
                    load_woc(kc)
                att_block(3)

                # residual rows, bf16, into dead K_sb space
                xs = K_sb  # [P, 2, T] view: (p, isub, rcol)
                for isub in range(2):
                    nc.sync.dma_start(xs[:, isub, :],
                                      x_seq[P * isub:P * (isub + 1), :])

                nc.gpsimd.collective_compute(
                    "AllToAll", Alu.bypass, replica_groups=groups,
                    ins=[a2a_in[:].opt()], outs=[a2a_out[:].opt()])

                # ============== output projection (sequence-sharded) =====
                stk0.close()  # free attention SBUF + PSUM pools
                with contextlib.ExitStack() as stk:
                    opool = stk.enter_context(tc.tile_pool(name="osb", bufs=8))
                    ops = stk.enter_context(
                        tc.tile_pool(name="o_ps", bufs=8, space="PSUM"))
                    # att_all reuses Q_sb's SBUF (dead after last scores)
                    att_all = Q_sb[:].rearrange("p a (c i) -> p (a c) i",
                                                i=TS_C)
                    avf = a2a_out[:].rearrange("c d i -> (c d) i")
                    for cc in range(n_cores):
                        nc.sync.dma_start(
                            att_all[:, 2 * cc:2 * cc + 2, :],
                            avf[DL * cc:DL * (cc + 1), :]
                            .rearrange("(a p) i -> p a i", p=P))
                    outp = [ops.tile([P, 512], f32, tag="om", name=f"om{t}")
                            for t in range(8)]
                    for kc in range(KC):
                        for isub in range(2):
                            for rg in range(RG):
                                t = 4 * isub + rg
                                nc.tensor.matmul(
                                    outp[t][:],
                                    att_all[:, kc, P * isub:P * (isub + 1)],
                                    wo_chunks[kc][:, rg, :],
                                    start=(kc == 0), stop=(kc == KC - 1))
                                if kc == KC - 1:
                                    rsl = slice(512 * rg, 512 * (rg + 1))
                                    osb = opool.tile([P, 512], f32,
                                                     tag="osb",
                                                     name=f"osb{t}")
                                    nc.vector.tensor_tensor(
                                        osb[:], outp[t][:],
                                        xs[:, isub, rsl], Alu.add)
                                    eng = nc.sync if t % 2 == 0 else nc.scalar
                                    eng.dma_start(
                                        out_rows[P * isub:P * (isub + 1),
                                                 rsl], osb[:])

    nc.compile()
    return nc


# --------------------------------------------------------------------------
# host-side prep / entry point
# --------------------------------------------------------------------------
def prepare_inputs(x, cos, sin, ln_w, Wq, Wk, Wv, Wo, n_cores, heads_per_core):
    import ml_dtypes
    bf16 = ml_dtypes.bfloat16
    DH = 128
    DL = heads_per_core * DH
    x = np.ascontiguousarray(np.asarray(x, dtype=np.float32))
    T, D = x.shape
    KC = D // DH
    TS_C = T // n_cores
    cosT = np.ascontiguousarray(np.asarray(cos, np.float32).T.astype(bf16))
    sinmT = np.asarray(sin, np.float32).T.copy()
    sinmT[64:, :] *= -1.0  # sign fold for the rotate-half swap trick
    sinmT = np.ascontiguousarray(sinmT.astype(bf16))
    lnw = np.asarray(ln_w, np.float32)
    xT = np.ascontiguousarray(x.T.astype(bf16))
    x_rows = np.ascontiguousarray(x.astype(bf16))

    def pretile_qkv(W, cols):
        # rows j of W (out dims), ln_w folded; SBUF layout [P, KC*DL]
        arr = (np.asarray(W, np.float32)[cols, :] * lnw[None, :]).T  # (D, DL)
        return np.ascontiguousarray(
            arr.reshape(KC, DH, DL).transpose(1, 0, 2).reshape(DH, KC * DL)
            .astype(bf16))

    # Wo full, pretiled: element (p, kc, rg, j) = Wo.T[128kc+p, 512rg+j]
    woT = np.asarray(Wo, np.float32).T  # (D, D) = (d_in, d_out)
    wo_t = np.ascontiguousarray(
        woT.reshape(KC, DH, 4, 512).transpose(1, 0, 2, 3)
        .reshape(DH, KC * 4 * 512).astype(bf16))

    in_maps = []
    for c in range(n_cores):
        cols = slice(c * DL, (c + 1) * DL)
        rows = slice(c * TS_C, (c + 1) * TS_C)
        in_maps.append({
            "xT": xT,
            "x_rows": x_rows,
            "x_seq": x_rows[rows, :],
            "wq_t": pretile_qkv(Wq, cols),
            "wk_t": pretile_qkv(Wk, cols),
            "wv_t": pretile_qkv(Wv, cols),
            "wo_t": wo_t,
            "cosT": cosT,
            "sinmT": sinmT,
        })
    return in_maps


_NC_CACHE = {}


def kernel(x, cos, sin, attention_mask, ln_w, Wq, Wk, Wv, Wo,
           _trace=False, _trace_cores=None):
    from concourse.bass_utils import run_bass_kernel_spmd

    cfg = CFG_FULL
    key = tuple(sorted(cfg.items()))
    if key not in _NC_CACHE:
        _NC_CACHE[key] = build_nc(**cfg)
    nc = _NC_CACHE[key]
    n_cores = cfg["n_cores"]
    in_maps = prepare_inputs(x, cos, sin, ln_w, Wq, Wk, Wv, Wo,
                             n_cores, cfg["heads_per_core"])
    res = run_bass_kernel_spmd(nc, in_maps, core_ids=list(range(n_cores)),
                               trace=_trace, trace_cores=_trace_cores)
    out = np.concatenate(
        [res.results[c]["out_rows"] for c in range(n_cores)], axis=0)
    kernel.last_result = res
    return out


# revision 36
# speedup vs baseline: 1.2078x; 1.2078x over previous
"""Trainium2 Bass kernel for a dense-transformer attention block.

Reference semantics (T=2048, D=2048, 16 heads, d_h=128):
    h = RMSNorm(x) * ln_w
    q,k,v = h @ W{q,k,v}.T  -> (n_h, T, d_h);  RoPE(q, k)
    att = softmax(causal(q k^T / sqrt(d_h))) @ v
    out = x + att @ Wo.T          (attention_mask is all-ones per spec)

Distribution: head-parallel over 8 cores (2 heads/core) for QKV+attention;
the output is SEQUENCE-sharded: one AllToAll redistributes att^T so each
core holds all 2048 att rows for its own 256 output timesteps, then each
core runs the FULL Wo against its timestep slice (~8x fewer collective
bytes than all-gathering att).

Engine/queue plan per core (DMAs execute synchronously on their issuing
queue at ~190GB/s, so the two DMA-capable queues are scheduled explicitly):
  sync   : x^T stream (block-major so block 0 lands first), wq/wk, then
           a2a stores / Wo even chunks / output stores.
  scalar : rope tables, wv, per-block r-transposes + rope half-swaps, exps
           (compute), Wo odd chunks.
  gpsimd : ONLY mask setup + collective triggers (a trigger blocks the
           gpsimd queue until the collective completes).
  r-chain: per 512-block B, x^2 on DVE, column sums via ones-vector PE
           matmuls, [1,512] bit-trick rsqrt Newton on DVE; r broadcast
           across partitions with a ones-column PE matmul and folded into
           the RoPE tables; r transposed to per-partition scalars for V
           with 4 tiny DMAs.
  QKV    : bf16 matmuls (ln_w folded into weights on host); Q/K PSUM
           drained raw so the PE never waits on r; RoPE in place on DVE
           with a sign-folded sin table + partition-half-swap DMA.
  att    : causal attention, scores transposed (S^T[j,i]); i-range sliced
           to skip fully-masked diagonal work; row sums via ones-vector
           matmuls; exp on ScalarE; 1/sum broadcast via ones-column matmul.
  Wo     : out[i,:] = att_all^T @ Wo, weights streamed during attention on
           spread queues; residual from a bf16 x slice preloaded into dead
           K_sb space; drains inline with the last accumulation pass.
Host assembles out = concat(out_rows, axis=0).
"""

import contextlib
import math

import numpy as np

EPS = 1e-5
NEG = -1.0e30

CFG_FULL = dict(T=2048, D=2048, n_cores=8, heads_per_core=2)


# --------------------------------------------------------------------------
# device program
# --------------------------------------------------------------------------
def build_nc(T, D, n_cores, heads_per_core):
    import concourse.mybir as mybir
    import concourse.tile as tile
    from concourse import bacc

    DH = 128                      # head dim (hard-wired into layout)
    P = 128                       # partitions
    NH = heads_per_core
    DL = NH * DH                  # local width (q/k/v columns per core)
    KC = D // P                   # 128-contraction chunks over d_model
    NTS = T // P                  # 128-wide t subtiles
    TS_C = T // n_cores           # output timesteps per core (256)
    RG = 4                        # Wo column groups of 512
    f32 = mybir.dt.float32
    bf16 = mybir.dt.bfloat16
    i32 = mybir.dt.int32

    nc = bacc.Bacc("TRN2", target_bir_lowering=False, debug=False,
                   num_devices=n_cores)

    # ---- I/O ----
    xT = nc.dram_tensor("xT", [D, T], bf16, kind="ExternalInput").ap()
    x_seq = nc.dram_tensor("x_seq", [TS_C, D], bf16, kind="ExternalInput").ap()
    wq_t = nc.dram_tensor("wq_t", [P, KC * DL], bf16, kind="ExternalInput").ap()
    wk_t = nc.dram_tensor("wk_t", [P, KC * DL], bf16, kind="ExternalInput").ap()
    wv_t = nc.dram_tensor("wv_t", [P, KC * DL], bf16, kind="ExternalInput").ap()
    wo_t = nc.dram_tensor("wo_t", [P, KC * RG * 512], bf16,
                          kind="ExternalInput").ap()
    cosT = nc.dram_tensor("cosT", [DH, T], bf16, kind="ExternalInput").ap()
    sinmT = nc.dram_tensor("sinmT", [DH, T], bf16, kind="ExternalInput").ap()
    out_rows = nc.dram_tensor("out_rows", [TS_C, D], f32,
                              kind="ExternalOutput").ap()

    Act = mybir.ActivationFunctionType
    Alu = mybir.AluOpType
    inv_sqrt_dh = 1.0 / math.sqrt(DH)
    MAGIC = 0x5F3759DF
    groups = [list(range(n_cores))]

    with tile.TileContext(nc) as tc, \
            tc.tile_pool(name="persist", bufs=1) as persist:
        # ---------------- long-lived tensors ----------------
        Q_sb = persist.tile([P, NH, T], bf16, tag="Q_sb")
        K_sb = persist.tile([P, NH, T], bf16, tag="K_sb")
        V_sb = persist.tile([P, NTS, DL], bf16, tag="V_sb")
        rcol_sb = persist.tile([P, NTS], f32, tag="rcol_sb")
        rrow_sb = persist.tile([1, T], bf16, tag="rrow_sb")
        rrow_f32 = persist.tile([1, T], f32, tag="rrow_f32")
        ones_bf = persist.tile([P, 1], bf16, tag="ones_bf")
        ones_row = persist.tile([1, P], bf16, tag="ones_row")
        mask_sb = persist.tile([P, P], f32, tag="mask_sb")
        warm_sb = persist.tile([P, 128], bf16, tag="warm_sb")

        nc.vector.memset(ones_bf[:], 1.0)
        nc.vector.memset(ones_row[:], 1.0)
        nc.vector.memset(warm_sb[:], 0.0)

        with tc.tile_pool(name="dram", bufs=1, space="DRAM") as dram_pool:
            a2a_in = dram_pool.tile([n_cores, DL, TS_C], bf16, tag="a2ai",
                                    name="a2a_in")
            a2a_out = dram_pool.tile([n_cores, DL, TS_C], bf16, tag="a2ao",
                                     name="a2a_out")
            dummy_i = dram_pool.tile([n_cores, 128], bf16, tag="dmi",
                                     name="dummy_i")
            dummy_o = dram_pool.tile([n_cores, 128], bf16, tag="dmo",
                                     name="dummy_o")

            nc.gpsimd.memset(mask_sb[:], 0.0)
            # keep (0) where j <= i within the diagonal 128x128 block:
            # j = base + p, i = base + f  ->  keep f - p >= 0
            nc.gpsimd.affine_select(
                out=mask_sb[:], in_=mask_sb[:],
                pattern=[[1, P]], channel_multiplier=-1, base=0,
                compare_op=Alu.is_ge, fill=NEG)
            # warm the collective path (absorbs the cross-core entry
            # barrier + first-op setup while the DMA-bound prologue runs)
            nc.sync.dma_start(dummy_i[:], warm_sb[:n_cores, :])
            nc.gpsimd.collective_compute(
                "AllToAll", Alu.bypass, replica_groups=groups,
                ins=[dummy_i[:].opt()], outs=[dummy_o[:].opt()])

            # PE warmup: back-to-back dummy matmuls so the HAM clock gate
            # opens before the real work arrives
            with tc.tile_pool(name="warm_ps", bufs=1, space="PSUM") as wmps:
                wps = wmps.tile([P, 128], f32, tag="wm")
                for _ in range(24):
                    nc.tensor.matmul(wps[:], warm_sb[:], warm_sb[:],
                                     start=True, stop=True)

            with contextlib.ExitStack() as stk_wo:
                wopool = stk_wo.enter_context(tc.tile_pool(name="wo", bufs=6))
                stk0 = stk_wo.enter_context(contextlib.ExitStack())
                _p = lambda *a, **kw: stk0.enter_context(tc.tile_pool(*a, **kw))
                wpool = _p(name="wqkv", bufs=1)
                cspool = _p(name="cs_raw", bufs=1)
                xpool = _p(name="xk", bufs=1)
                sqpool = _p(name="sqs", bufs=8)
                rpool = _p(name="rtmp", bufs=1)
                ropool = _p(name="rope", bufs=2)
                ptpool = _p(name="ptp", bufs=3)
                finpool = _p(name="fin", bufs=2)
                pps = _p(name="proj_ps", bufs=3, space="PSUM")
                stps = _p(name="st_ps", bufs=2, space="PSUM")
                avps = _p(name="av_ps", bufs=1, space="PSUM")
                rowps = _p(name="row_ps", bufs=1, space="PSUM")
                bcps = _p(name="bc_ps", bufs=1, space="PSUM")

                # ---- initial loads: explicit 2-queue schedule ----
                wq_sb = wpool.tile([P, KC, DL], bf16, tag="wq")
                wk_sb = wpool.tile([P, KC, DL], bf16, tag="wk")
                wv_sb = wpool.tile([P, KC, DL], bf16, tag="wv")
                xk = [xpool.tile([P, T], bf16, tag=f"xk{kc}", name=f"xk{kc}")
                      for kc in range(KC)]
                cos_bf = cspool.tile([P, T], bf16, tag="cosb")
                sinm_bf = cspool.tile([P, T], bf16, tag="sinb")
                cos_r = cspool.tile([P, T], f32, tag="cos")
                sinm_r = cspool.tile([P, T], f32, tag="sin")
                nc.scalar.dma_start(cos_bf[:], cosT)
                nc.scalar.dma_start(sinm_bf[:], sinmT)
                nc.scalar.dma_start(wv_sb[:], wv_t.rearrange("p (kc j) -> p kc j", j=DL))
                nc.sync.dma_start(wq_sb[:], wq_t.rearrange("p (kc j) -> p kc j", j=DL))

                def load_x_block(B):
                    tb = slice(512 * B, 512 * B + 512)
                    for kc in range(KC):
                        nc.sync.dma_start(xk[kc][:, tb],
                                          xT[P * kc:P * (kc + 1), tb])

                load_x_block(0)
                nc.sync.dma_start(wk_sb[:], wk_t.rearrange("p (kc j) -> p kc j", j=DL))
                for B in range(1, 4):
                    load_x_block(B)

                # ---- per-block r = rsqrt(mean(x^2)+eps) ----
                mrow = rpool.tile([1, 512], f32, tag="mrow")
                rirow = rpool.tile([1, 512], i32, tag="rirow")
                tnrow = rpool.tile([1, 512], f32, tag="tnrow")

                def r_sums(B):
                    tb = slice(512 * B, 512 * B + 512)
                    srow = bcps.tile([1, 512], f32, tag="bc",
                                     name=f"srow{B}")
                    for kc in range(KC):
                        sq = sqpool.tile([P, 512], bf16, tag="sq")
                        nc.vector.tensor_tensor(sq[:], xk[kc][:, tb],
                                                xk[kc][:, tb], Alu.mult)
                        nc.tensor.matmul(srow[:], ones_bf[:], sq[:],
                                         start=(kc == 0), stop=(kc == KC - 1))
                    # r = rsqrt(sum/D + eps): bit-trick seed + 2 Newton
                    rr = rrow_sb[0:1, tb]
                    nc.vector.tensor_scalar(mrow[:], srow[:], 1.0 / D, EPS,
                                            Alu.mult, Alu.add)
                    nc.vector.tensor_scalar(rirow[:], mrow[:].bitcast(i32), 1,
                                            None, Alu.arith_shift_right)
                    nc.vector.tensor_scalar(rirow[:], rirow[:], -1, MAGIC,
                                            Alu.mult, Alu.add)
                    rrv = rirow[:].bitcast(f32)
                    for _ in range(2):
                        nc.vector.tensor_tensor(tnrow[:], rrv, rrv, Alu.mult)
                        nc.vector.tensor_tensor(tnrow[:], tnrow[:], mrow[:],
                                                Alu.mult)
                        nc.vector.tensor_scalar(tnrow[:], tnrow[:], -0.5, 1.5,
                                                Alu.mult, Alu.add)
                        nc.vector.tensor_tensor(rrv, rrv, tnrow[:], Alu.mult)
                    nc.vector.tensor_copy(rr, rrv)
                    nc.vector.tensor_copy(rrow_f32[0:1, tb], rrv)
                    # transpose r to per-partition scalars for the V scaling
                    for s in range(4):
                        i = 4 * B + s
                        nc.scalar.dma_start(
                            rcol_sb[:, i:i + 1],
                            rrow_f32[0:1, P * i:P * (i + 1)])

                def r_tables(B):
                    tb = slice(512 * B, 512 * B + 512)
                    rps = bcps.tile([P, 512], f32, tag="bc", name=f"rbc{B}")
                    nc.tensor.matmul(rps[:], ones_row[:], rrow_sb[0:1, tb],
                                     start=True, stop=True)
                    nc.vector.tensor_tensor(cos_r[:, tb], cos_bf[:, tb],
                                            rps[:], Alu.mult)
                    nc.vector.tensor_tensor(sinm_r[:, tb], sinm_bf[:, tb],
                                            rps[:], Alu.mult)

                # Wo weight chunks: stream during attention, spread queues
                wo_view = wo_t.rearrange("p (kc rg j) -> p kc rg j",
                                         kc=KC, rg=RG)
                wo_chunks = [None] * KC
                _wo_engs = [nc.sync, nc.scalar]

                def load_woc(kc):
                    woc = wopool.tile([P, RG, 512], bf16, tag="woc",
                                      name=f"woc{kc}")
                    _wo_engs[kc % 2].dma_start(woc[:], wo_view[:, kc, :, :])
                    wo_chunks[kc] = woc

                # ---- per-block QKV matmuls (PE decoupled from r) ----
                def qkv_mms(B):
                    tb = slice(512 * B, 512 * B + 512)
                    for qk, (w_sb, dst) in enumerate(((wq_sb, Q_sb),
                                                      (wk_sb, K_sb))):
                        for h in range(NH):
                            hs = slice(DH * h, DH * (h + 1))
                            qp = pps.tile([P, 512], f32, tag="proj",
                                          name=f"p{B}_{qk}_{h}")
                            for kc in range(KC):
                                nc.tensor.matmul(qp[:], w_sb[:, kc, hs],
                                                 xk[kc][:, tb],
                                                 start=(kc == 0),
                                                 stop=(kc == KC - 1))
                            # raw drain; RoPE comes later, in place
                            nc.vector.tensor_copy(dst[:, h, tb], qp[:])
                    for ts in range(4):
                        i = 4 * B + ts
                        tsl = slice(512 * B + P * ts, 512 * B + P * (ts + 1))
                        vp = pps.tile([P, 512], f32, tag="proj",
                                      name=f"pv{B}_{ts}")
                        for kc in range(KC):
                            nc.tensor.matmul(vp[:, :DL], xk[kc][:, tsl],
                                             wv_sb[:, kc, :], start=(kc == 0),
                                             stop=(kc == KC - 1))
                        nc.vector.tensor_scalar_mul(V_sb[:, i, :], vp[:, :DL],
                                                    rcol_sb[:, i:i + 1])

                def rope(B):
                    tb = slice(512 * B, 512 * B + 512)
                    for dst in (Q_sb, K_sb):
                        for h in range(NH):
                            # dst = dst*cos_r + swap64(dst*sinm_r), in place
                            tmp = ropool.tile([P, 512], bf16, tag="tmp")
                            tmp2 = ropool.tile([P, 512], bf16, tag="tmp2")
                            nc.vector.tensor_tensor(tmp[:], dst[:, h, tb],
                                                    sinm_r[:, tb], Alu.mult)
                            nc.scalar.dma_start(tmp2[0:64, :], tmp[64:128, :])
                            nc.scalar.dma_start(tmp2[64:128, :], tmp[0:64, :])
                            nc.vector.tensor_tensor(dst[:, h, tb],
                                                    dst[:, h, tb],
                                                    cos_r[:, tb], Alu.mult)
                            nc.vector.tensor_tensor(dst[:, h, tb],
                                                    dst[:, h, tb], tmp2[:],
                                                    Alu.add)

                def att_block(B):
                    for h in range(NH):
                        hs = slice(DH * h, DH * (h + 1))
                        av = avps.tile([P, 512], f32, tag="av",
                                       name=f"av{B}_{h}")
                        ssum = rowps.tile([1, 512], f32, tag="row",
                                          name=f"ss{B}_{h}")
                        Jmax = 4 * B + 3
                        for J in range(Jmax + 1):
                            r = J - 4 * B  # >=0 on the diagonal 512-block
                            lo = max(0, 128 * r)  # live i-range start
                            isl = slice(512 * B + lo, 512 * B + 512)
                            st = stps.tile([P, 512], f32, tag="st",
                                           name=f"st{B}_{h}_{J}")
                            nc.tensor.matmul(st[:, lo:],
                                             K_sb[:, h, P * J:P * (J + 1)],
                                             Q_sb[:, h, isl],
                                             start=True, stop=True)
                            if r >= 0:
                                nc.vector.tensor_tensor(
                                    st[:, lo:lo + P], st[:, lo:lo + P],
                                    mask_sb[:], Alu.add)
                            pt = ptpool.tile([P, 512], bf16, tag="pt")
                            nc.scalar.activation(pt[:, lo:], st[:, lo:],
                                                 Act.Exp, scale=inv_sqrt_dh)
                            nc.tensor.matmul(av[:, lo:], V_sb[:, J, hs],
                                             pt[:, lo:], start=(J == 0),
                                             stop=(J == Jmax),
                                             skip_group_check=True)
                            nc.tensor.matmul(ssum[:, lo:], ones_bf[:],
                                             pt[:, lo:], start=(J == 0),
                                             stop=(J == Jmax),
                                             skip_group_check=True)
                        rinv = finpool.tile([1, 512], f32, tag="rinv")
                        nc.vector.reciprocal_approx_fast(rinv[:], ssum[:])
                        rinv_bf = finpool.tile([1, 512], bf16, tag="rinvb")
                        nc.vector.tensor_copy(rinv_bf[:], rinv[:])
                        rb = bcps.tile([P, 512], f32, tag="bc",
                                       name=f"rb{B}_{h}")
                        nc.tensor.matmul(rb[:], ones_row[:], rinv_bf[:],
                                         start=True, stop=True)
                        rb_sb = finpool.tile([P, 512], f32, tag="rbsb")
                        nc.scalar.activation(rb_sb[:], rb[:], Act.Copy)
                        att = finpool.tile([P, 512], bf16, tag="att")
                        nc.vector.tensor_tensor(att[:], av[:], rb_sb[:],
                                                Alu.mult)
                        # store to the AllToAll source: dest core 2B+c' gets
                        # i-window att[:, 256c' : 256c'+256]
                        nc.sync.dma_start(
                            a2a_in[2 * B:2 * B + 2, DH * h:DH * (h + 1), :]
                            .rearrange("c p i -> p c i"),
                            att[:])

                r_sums(0)
                qkv_mms(0)
                r_tables(0)
                rope(0)
                r_sums(1)
                qkv_mms(1)
                r_tables(1)
                rope(1)
                load_woc(0)
                load_woc(1)
                att_block(0)
                r_sums(2)
                qkv_mms(2)
                r_tables(2)
                rope(2)
                for kc in range(2, 6):
                    load_woc(kc)
                att_block(1)
                r_sums(3)
                qkv_mms(3)
                r_tables(3)
                rope(3)
                for kc in range(6, 11):
                    load_woc(kc)
                att_block(2)
                for kc in range(11, KC):
                    load_woc(kc)
                att_block(3)

                # residual rows, bf16, into dead K_sb space
                xs = K_sb  # [P, 2, T] view: (p, isub, rcol)
                for isub in range(2):
                    nc.sync.dma_start(xs[:, isub, :],
                                      x_seq[P * isub:P * (isub + 1), :])

                nc.gpsimd.collective_compute(
                    "AllToAll", Alu.bypass, replica_groups=groups,
                    ins=[a2a_in[:].opt()], outs=[a2a_out[:].opt()])

                # ============== output projection (sequence-sharded) =====
                stk0.close()  # free attention SBUF + PSUM pools
                with contextlib.ExitStack() as stk:
                    opool = stk.enter_context(tc.tile_pool(name="osb", bufs=8))
                    ops = stk.enter_context(
                        tc.tile_pool(name="o_ps", bufs=8, space="PSUM"))
                    # att_all reuses Q_sb's SBUF (dead after last scores)
                    att_all = Q_sb[:].rearrange("p a (c i) -> p (a c) i",
                                                i=TS_C)
                    avf = a2a_out[:].rearrange("c d i -> (c d) i")
                    for cc in range(n_cores):
                        eng = nc.sync if cc % 2 == 0 else nc.scalar
                        eng.dma_start(
                            att_all[:, 2 * cc:2 * cc + 2, :],
                            avf[DL * cc:DL * (cc + 1), :]
                            .rearrange("(a p) i -> p a i", p=P))
                    outp = [ops.tile([P, 512], f32, tag="om", name=f"om{t}")
                            for t in range(8)]
                    for kc in range(KC):
                        for isub in range(2):
                            for rg in range(RG):
                                t = 4 * isub + rg
                                nc.tensor.matmul(
                                    outp[t][:],
                                    att_all[:, kc, P * isub:P * (isub + 1)],
                                    wo_chunks[kc][:, rg, :],
                                    start=(kc == 0), stop=(kc == KC - 1))
                                if kc == KC - 1:
                                    rsl = slice(512 * rg, 512 * (rg + 1))
                                    osb = opool.tile([P, 512], f32,
                                                     tag="osb",
                                                     name=f"osb{t}")
                                    nc.vector.tensor_tensor(
                                        osb[:], outp[t][:],
                                        xs[:, isub, rsl], Alu.add)
                                    eng = nc.sync if t % 2 == 0 else nc.scalar
                                    eng.dma_start(
                                        out_rows[P * isub:P * (isub + 1),
                                                 rsl], osb[:])

    nc.compile()
    return nc


# --------------------------------------------------------------------------
# host-side prep / entry point
# --------------------------------------------------------------------------
def prepare_inputs(x, cos, sin, ln_w, Wq, Wk, Wv, Wo, n_cores, heads_per_core):
    import ml_dtypes
    bf16 = ml_dtypes.bfloat16
    DH = 128
    DL = heads_per_core * DH
    x = np.ascontiguousarray(np.asarray(x, dtype=np.float32))
    T, D = x.shape
    KC = D // DH
    TS_C = T // n_cores
    cosT = np.ascontiguousarray(np.asarray(cos, np.float32).T.astype(bf16))
    sinmT = np.asarray(sin, np.float32).T.copy()
    sinmT[64:, :] *= -1.0  # sign fold for the rotate-half swap trick
    sinmT = np.ascontiguousarray(sinmT.astype(bf16))
    lnw = np.asarray(ln_w, np.float32)
    xT = np.ascontiguousarray(x.T.astype(bf16))
    x_bf = np.ascontiguousarray(x.astype(bf16))

    def pretile_qkv(W, cols):
        # rows j of W (out dims), ln_w folded; SBUF layout [P, KC*DL]
        arr = (np.asarray(W, np.float32)[cols, :] * lnw[None, :]).T  # (D, DL)
        return np.ascontiguousarray(
            arr.reshape(KC, DH, DL).transpose(1, 0, 2).reshape(DH, KC * DL)
            .astype(bf16))

    # Wo full, pretiled: element (p, kc, rg, j) = Wo.T[128kc+p, 512rg+j]
    woT = np.asarray(Wo, np.float32).T  # (D, D) = (d_in, d_out)
    wo_t = np.ascontiguousarray(
        woT.reshape(KC, DH, 4, 512).transpose(1, 0, 2, 3)
        .reshape(DH, KC * 4 * 512).astype(bf16))

    in_maps = []
    for c in range(n_cores):
        cols = slice(c * DL, (c + 1) * DL)
        rows = slice(c * TS_C, (c + 1) * TS_C)
        in_maps.append({
            "xT": xT,
            "x_seq": np.ascontiguousarray(x_bf[rows, :]),
            "wq_t": pretile_qkv(Wq, cols),
            "wk_t": pretile_qkv(Wk, cols),
            "wv_t": pretile_qkv(Wv, cols),
            "wo_t": wo_t,
            "cosT": cosT,
            "sinmT": sinmT,
        })
    return in_maps


_NC_CACHE = {}


def kernel(x, cos, sin, attention_mask, ln_w, Wq, Wk, Wv, Wo,
           _trace=False, _trace_cores=None):
    from concourse.bass_utils import run_bass_kernel_spmd

    cfg = CFG_FULL
    key = tuple(sorted(cfg.items()))
    if key not in _NC_CACHE:
        _NC_CACHE[key] = build_nc(**cfg)
    nc = _NC_CACHE[key]
    n_cores = cfg["n_cores"]
    in_maps = prepare_inputs(x, cos, sin, ln_w, Wq, Wk, Wv, Wo,
                             n_cores, cfg["heads_per_core"])
    res = run_bass_kernel_spmd(nc, in_maps, core_ids=list(range(n_cores)),
                               trace=_trace, trace_cores=_trace_cores)
    out = np.concatenate(
        [res.results[c]["out_rows"] for c in range(n_cores)], axis=0)
    kernel.last_result = res
    return out
